# revision 28
# baseline (speedup 1.0000x reference)
"""Deformable Conv2D (DCNv2-style) on 8 Trainium2 NeuronCores.

Strategy (data-parallel over batch, one sample per core):
  conv-first reformulation:  out[f,j] = sum_kk sum_corner w_corner[kk,j] * Y_kk[f, p_corner(kk,j)]
  where Y_kk = W[:,:,kk] @ x  (plain matmul over all spatial positions).

  Sampling uses a per-tap DRAM table TC[kk] whose row t packs the 4 bilinear
  corner pixel-vectors [Y(t-65) | Y(t-64) | Y(t-1) | Y(t)] (bf16, 1 KB), so a
  single dma_gather descriptor per (tap, output position) fetches all four
  corners.  The table is produced directly by the TensorEngine: for each
  128-row tile and each of the 4 column slots, a matmul with a shifted x tile
  as the stationary operand emits Y^T rows already in table layout.

  Bilinear/mask/validity weights are folded into 4 per-position corner
  weights on the host.  The combine runs as large DVE tensor_tensor ops in
  2x mode: weights are stored pair-duplicated ([w,w] adjacent) so the
  broadcast AP keeps a packed innermost [1,2] dim; corner partials reduce
  with bf16 adds into a bf16 accumulator (host upcasts to f32).  All psum
  evictions run on ACT so the Vector engine stays dedicated to the combine;
  table groups build in widths 1/2/3/3 so gathers start early; the last tap
  gathers in halves so its combine and the output DMA overlap.  The output
  stays position-major on device; the host does the final [j,f] -> [f,j]
  transpose.

Shapes (hardcoded per spec): x (8,128,64,64) f32, offset (8,18,64,64),
mask (8,9,64,64), weight (128,128,3,3), out (8,128,64,64) f32.
"""

import numpy as np
import ml_dtypes
from contextlib import ExitStack

import concourse.bass as bass
import concourse.bacc as bacc
import concourse.tile as tile
from concourse import mybir
from concourse.bass_utils import run_bass_kernel_spmd

B, C, H, W = 8, 128, 64, 64
F = 128
KH = KW = 3
KK = KH * KW
HW = H * W  # 4096
NP = 128
NII = HW // NP  # 32 position blocks
NTT = 33  # table row tiles (t in [0, 4224)); gather uses rows [0, 4160]
TROWS = NTT * NP
TCOLS = 4 * F  # 512
# column-slot source shifts: TC[t] = [Y(t-65) | Y(t-64) | Y(t-1) | Y(t)]
SLOT_SHIFT = (-65, -64, -1, 0)
XPAD_LO = 65  # x padding so shifted tiles never index out of range
XPAD = XPAD_LO + TROWS + 64  # padded x columns

# gather call schedule: (base_tap, [(tap, ii0, nii), ...]); idx values are
# stored relative to base_tap's table rows (int16-safe: max 1*TROWS+4160).
# 6144-idx calls amortize the ~9us fixed SWDGE cost per dma_gather; the
# final 2048-idx call keeps the critical tail short.
G_CALLS = [(k, [(k, 0, 32)]) for k in range(8)] + [
    (8, [(8, 0, 16)]),
    (8, [(8, 16, 8)]),
    (8, [(8, 24, 8)]),
]

BF16 = mybir.dt.bfloat16
F32 = mybir.dt.float32
I16 = mybir.dt.int16


def _prep_indices_weights(offset, mask):
    """Per-sample host prep. offset [18,H,W], mask [9,H,W] ->
    idx int16 [128, KK*256], wts bf16 [128, KK*NII*4*2]."""
    off = offset.reshape(KK, 2, H, W)
    dy, dx = off[:, 0], off[:, 1]
    ki, kj = np.meshgrid(np.arange(KH), np.arange(KW), indexing="ij")
    ki = ki.reshape(KK, 1, 1).astype(np.float32)
    kj = kj.reshape(KK, 1, 1).astype(np.float32)
    base_y = (np.arange(H, dtype=np.float32) - 1.0)[None, :, None] + ki
    base_x = (np.arange(W, dtype=np.float32) - 1.0)[None, None, :] + kj
    py = base_y + dy
    px = base_x + dx
    y0 = np.floor(py)
    x0 = np.floor(px)
    ly = (py - y0).astype(np.float32)
    lx = (px - x0).astype(np.float32)
    hy = 1.0 - ly
    hx = 1.0 - lx
    y0i = y0.astype(np.int64)
    x0i = x0.astype(np.int64)

    vy0 = (y0i >= 0) & (y0i < H)
    vy1 = (y0i + 1 >= 0) & (y0i + 1 < H)
    vx0 = (x0i >= 0) & (x0i < W)
    vx1 = (x0i + 1 >= 0) & (x0i + 1 < W)

    m = mask.reshape(KK, H, W)
    # corner order matches table slots: (y0,x0) (y0,x1) (y1,x0) (y1,x1)
    w4 = np.stack(
        [
            (hy * hx * m * (vy0 & vx0)).reshape(KK, HW),
            (hy * lx * m * (vy0 & vx1)).reshape(KK, HW),
            (ly * hx * m * (vy1 & vx0)).reshape(KK, HW),
            (ly * lx * m * (vy1 & vx1)).reshape(KK, HW),
        ],
        axis=1,
    ).astype(np.float32)  # [KK, 4, HW]

    flat = np.clip(y0i * W + x0i + 65, 0, HW + 64).reshape(KK, HW)

    # idx: per gather call, ordinals o wrapped -> [o%16, o//16], replicated
    # to 128 partitions (dma_gather consumes idxs from each 16-partition
    # group).  Rows of taps beyond the call's base tap get +TROWS offsets.
    cols = []
    for kk0, parts in G_CALLS:
        fc = np.concatenate(
            [
                flat[kk][ii0 * NP : (ii0 + nii) * NP] + (kk - kk0) * TROWS
                for kk, ii0, nii in parts
            ]
        )
        wrapped = fc.astype(np.int16).reshape(-1, 16).T
        cols.append(np.tile(wrapped, (8, 1)))
    idx_dev = np.concatenate(cols, axis=1)  # [128, KK*256]

    # wts: [p, kk, ii, cr, 2] with value w4[kk, cr, ii*128+p] pair-duplicated
    wtsp = w4.reshape(KK, 4, NII, NP).transpose(3, 0, 2, 1)  # [p, kk, ii, cr]
    wts_dev = np.repeat(
        np.ascontiguousarray(wtsp).astype(ml_dtypes.bfloat16)[..., None], 2, axis=-1
    )
    return idx_dev, wts_dev.reshape(NP, KK * NII * 4 * 2)


def _split_overfull_waits(nc):
    """This walrus build accepts 1 sync-wait per instruction (2 for EVSEM).
    Move extras onto preceding same-engine NoOps."""
    for f in nc.m.functions:
        for bb in f.blocks:
            new_list = []
            for ins in bb.instructions:
                si = ins.sync_info
                waits = list(si.on_wait) if si and si.on_wait else []
                cap = 2 if isinstance(ins, mybir.InstEventSemaphore) else 1
                if len(waits) > cap:
                    extra, keep = waits[:-cap], waits[-cap:]
                    for k, w in enumerate(extra):
                        nop = mybir.InstNoOp(
                            name=f"{ins.name}_waitsplit{k}",
                            sync_info=mybir.SyncInfo(on_wait=[w], on_update=[]),
                            bass_nofuse=True,
                            engine=ins.engine,
                        )
                        new_list.append(nop)
                        nc.register_instruction(nop, overwrite=True)
                    si.on_wait = keep
                new_list.append(ins)
            bb.instructions[:] = new_list


def _build_nc():
    nc = bacc.Bacc(None, target_bir_lowering=False, debug=False)
    x_d = nc.dram_tensor("x", [NP, XPAD], BF16, kind="ExternalInput")
    wt_d = nc.dram_tensor("wt", [NP, KK * F], BF16, kind="ExternalInput")
    idx_d = nc.dram_tensor("idx", [NP, KK * 256], I16, kind="ExternalInput")
    wts_d = nc.dram_tensor("wts", [NP, KK * NII * 4 * 2], BF16, kind="ExternalInput")
    out_d = nc.dram_tensor("out", [NP, HW], BF16, kind="ExternalOutput")
    tbl_d = nc.dram_tensor("tbl", [KK, TROWS, TCOLS], BF16, kind="Internal")

    TBL_KK = TROWS * TCOLS

    with tile.TileContext(nc) as tc, ExitStack() as ctx:
        cpool = ctx.enter_context(tc.tile_pool(name="const", bufs=1))
        tcst_pool = ctx.enter_context(tc.tile_pool(name="tcst", bufs=4))
        tcst2_pool = ctx.enter_context(tc.tile_pool(name="tcst2", bufs=2))
        gpool = ctx.enter_context(tc.tile_pool(name="gat", bufs=3))
        mpool = ctx.enter_context(tc.tile_pool(name="mul", bufs=1))
        spool = ctx.enter_context(tc.tile_pool(name="sum", bufs=1))
        accpool = ctx.enter_context(tc.tile_pool(name="acc", bufs=1))
        pspool = ctx.enter_context(tc.tile_pool(name="ps", bufs=2, space="PSUM"))

        x_sb = cpool.tile([NP, XPAD], BF16)
        wt_sb = cpool.tile([NP, KK * F], BF16)
        idx_sb = cpool.tile([NP, KK * 256], I16)
        wts_sb = cpool.tile([NP, KK, NII, 4, 2], BF16)
        acc_sb = accpool.tile([NP, NII, NP], BF16)

        # split x load so tap-0's first table matmuls start sooner
        nc.sync.dma_start(x_sb[:, 0:1152], x_d[:, 0:1152])
        nc.sync.dma_start(wt_sb[:], wt_d[:])
        nc.sync.dma_start(x_sb[:, 1152:], x_d[:, 1152:])
        nc.sync.dma_start(idx_sb[:], idx_d[:])
        nc.sync.dma_start(wts_sb[:].rearrange("p a b c d -> p (a b c d)"), wts_d[:])

        # ---- Stage A: build TC tables.
        # per (group, tt): stationary = shifted x tile; 4 slot matmuls
        # -> psum f32 -> ACT evict (cast bf16) into tcst staging
        # [q, (kk, slot, f)]; one DMA per (group, tt) writes the group's rows.
        # First group = 1 tap so gathers start early; rest in widths of 2.
        for lo, hi in ((0, 1), (1, 2), (2, 3), (3, 5), (5, 7), (7, 9)):
            nk = hi - lo
            if nk == 1:
                # width-1 groups: stage 2 tt per write DMA (fewer sync
                # dispatches + fewer sems on the dependent gather)
                for tt0 in range(0, NTT, 2):
                    npair = min(2, NTT - tt0)
                    tcst = tcst_pool.tile([NP, 2, 4, F], BF16, tag="tc1")
                    for ti in range(npair):
                        tt = tt0 + ti
                        ps = pspool.tile([NP, 4, 512], F32)
                        for s in range(4):
                            xoff = XPAD_LO + tt * NP + SLOT_SHIFT[s]
                            nc.tensor.matmul(
                                ps[:, s, 0:F],
                                x_sb[:, xoff : xoff + NP],
                                wt_sb[:, lo * F : hi * F],
                                start=True,
                                stop=True,
                            )
                        # tap-0 evictions ride on DVE (provably idle: the
                        # first combine cannot start before tap-0's gather
                        # finishes); ACT handles later groups.
                        if lo == 0:
                            nc.vector.tensor_copy(tcst[:, ti], ps[:, :, 0:F])
                        else:
                            nc.scalar.copy(tcst[:, ti], ps[:, :, 0:F])
                    dst = bass.AP(
                        tbl_d,
                        lo * TBL_KK + tt0 * NP * TCOLS,
                        [[TCOLS, NP], [NP * TCOLS, npair], [1, TCOLS]],
                    )
                    nc.sync.dma_start(dst, tcst[:, 0:npair])
                continue
            # width-2 groups: stage 4 tt, then one write DMA per tap
            for tt0 in range(0, NTT, 4):
                nb = min(4, NTT - tt0)
                tcst = tcst2_pool.tile([NP, 4, 2, 4, F], BF16, tag="tc2")
                for ti in range(nb):
                    tt = tt0 + ti
                    ps = pspool.tile([NP, 4, 512], F32)
                    for s in range(4):
                        xoff = XPAD_LO + tt * NP + SLOT_SHIFT[s]
                        nc.tensor.matmul(
                            ps[:, s, 0 : 2 * F],
                            x_sb[:, xoff : xoff + NP],
                            wt_sb[:, lo * F : hi * F],
                            start=True,
                            stop=True,
                        )
                    src = ps[:, :, 0 : 2 * F].rearrange("p s (k f) -> p s k f", k=2)
                    nc.scalar.copy(tcst[:, ti].rearrange("p k s f -> p s k f"), src)
                for kl in range(2):
                    dst = bass.AP(
                        tbl_d,
                        (lo + kl) * TBL_KK + tt0 * NP * TCOLS,
                        [[TCOLS, NP], [NP * TCOLS, nb], [1, TCOLS]],
                    )
                    nc.sync.dma_start(dst, tcst[:, 0:nb, kl])

        # ---- Stage B: gather + batched combine per G_CALLS schedule.
        def combine(kk, g_t, ii0, nii):
            """g_t [NP, nii, 4, F] gathered corners for positions
            (ii0..ii0+nii)*128; weighted-reduce into acc_sb."""
            w_ap = (
                wts_sb[:, kk, ii0 : ii0 + nii]
                .rearrange("p i c d -> p (i c) d")[:, :, None, :]
                .to_broadcast([NP, nii * 4, F // 2, 2])
            )
            if nii <= 16:
                # tail calls: separate mult output so the add tree reads
                # DVE-produced data (no DMA-sem waits -> shorter tail)
                m_tile = mpool.tile([NP, 16, 4, F], BF16, tag="m_t", name="m_tile")
                m_t = m_tile[:, :nii]
            else:
                m_t = g_t
            nc.vector.tensor_mul(
                m_t.rearrange("p i c (a b) -> p (i c) a b", b=2),
                g_t.rearrange("p i c (a b) -> p (i c) a b", b=2),
                w_ap,
            )
            g_t = m_t
            a_sl = acc_sb[:, ii0 : ii0 + nii]
            t01 = spool.tile([NP, NII, F], BF16, tag="t01")
            t23 = spool.tile([NP, NII, F], BF16, tag="t23")
            nc.vector.tensor_add(t01[:, :nii], g_t[:, :, 0], g_t[:, :, 1])
            nc.vector.tensor_add(t23[:, :nii], g_t[:, :, 2], g_t[:, :, 3])
            if kk == 0:
                nc.vector.tensor_add(a_sl, t01[:, :nii], t23[:, :nii])
            else:
                nc.vector.tensor_add(t01[:, :nii], t01[:, :nii], t23[:, :nii])
                nc.vector.tensor_add(a_sl, a_sl, t01[:, :nii])

        off = 0
        for ci, (kk0, parts) in enumerate(G_CALLS):
            nb = sum(nii for _, _, nii in parts)
            n = nb * NP
            g_full = gpool.tile([NP, NII, 4, F], BF16, tag="g_t")
            g_t = g_full[:, 0:nb]
            span = (max(kk for kk, _, _ in parts) - kk0) * TROWS + HW + 65
            src = bass.AP(tbl_d, kk0 * TBL_KK, [[TCOLS, span], [1, TCOLS]])
            nc.gpsimd.dma_gather(
                out_ap=g_t.rearrange("p i c f -> p i (c f)"),
                in_ap=src,
                idxs_ap=idx_sb[:, off : off + n // 16],
                num_idxs=n,
                num_idxs_reg=n,
                elem_size=TCOLS,
                single_packet=False,
            )
            off += n // 16
            pos = 0
            for kk, ii0, nii in parts:
                combine(kk, g_t[:, pos : pos + nii], ii0, nii)
                pos += nii
            if ci == len(G_CALLS) - 3:
                # all taps' combines for positions ii<16 are done
                nc.sync.dma_start(
                    out_d[:, 0 : NP * 16],
                    acc_sb[:, 0:16].rearrange("p i f -> p (i f)"),
                )
            elif ci == len(G_CALLS) - 2:
                nc.sync.dma_start(
                    out_d[:, NP * 16 : NP * 24],
                    acc_sb[:, 16:24].rearrange("p i f -> p (i f)"),
                )
            elif ci == len(G_CALLS) - 1:
                nc.sync.dma_start(
                    out_d[:, NP * 24 :],
                    acc_sb[:, 24:].rearrange("p i f -> p (i f)"),
                )

    nc.compile()
    _split_overfull_waits(nc)
    return nc


_NC_CACHE = {}


def _get_nc():
    if "nc" not in _NC_CACHE:
        _NC_CACHE["nc"] = _build_nc()
    return _NC_CACHE["nc"]


def _prep_x(xb):
    """x [C,H,W] f32 -> padded bf16 [128, XPAD]."""
    xp = np.zeros((C, XPAD), ml_dtypes.bfloat16)
    xp[:, XPAD_LO : XPAD_LO + HW] = xb.reshape(C, HW).astype(ml_dtypes.bfloat16)
    return xp


def kernel(x, offset, mask, weight, **run_kwargs):
    x = np.asarray(x, np.float32)
    offset = np.asarray(offset, np.float32)
    mask = np.asarray(mask, np.float32)
    weight = np.asarray(weight, np.float32)

    wt = np.transpose(weight.reshape(F, C, KK), (1, 2, 0)).reshape(C, KK * F)
    wt = np.ascontiguousarray(wt).astype(ml_dtypes.bfloat16)

    in_maps = []
    for b in range(B):
        idx_dev, wts_dev = _prep_indices_weights(offset[b], mask[b])
        in_maps.append(
            {
                "x": _prep_x(x[b]),
                "wt": wt,
                "idx": idx_dev,
                "wts": wts_dev,
            }
        )

    nc = _get_nc()
    res = run_bass_kernel_spmd(nc, in_maps, core_ids=list(range(8)), **run_kwargs)
    out = np.stack(
        [
            np.asarray(res.results[b]["out"])
            .astype(np.float32)
            .reshape(NP, NII, F)
            .transpose(2, 1, 0)
            .reshape(F, H, W)
            for b in range(B)
        ]
    )
    if run_kwargs:
        kernel.last_results = res
    return out


# revision 29
# speedup vs baseline: 1.1804x; 1.1804x over previous
"""Deformable Conv2D (DCNv2-style) on 8 Trainium2 NeuronCores.

Strategy (data-parallel over batch, one sample per core):
  conv-first reformulation:  out[f,j] = sum_kk sum_corner w_corner[kk,j] * Y_kk[f, p_corner(kk,j)]
  where Y_kk = W[:,:,kk] @ x  (plain matmul over all spatial positions).

  Sampling uses a per-tap DRAM table TC[kk] whose row t packs the 4 bilinear
  corner pixel-vectors [Y(t-65) | Y(t-64) | Y(t-1) | Y(t)] (bf16, 1 KB), so a
  single dma_gather descriptor per (tap, output position) fetches all four
  corners.  The table is produced directly by the TensorEngine: for each
  128-row tile and each of the 4 column slots, a matmul with a shifted x tile
  as the stationary operand emits Y^T rows already in table layout.

  Bilinear/mask/validity weights are folded into 4 per-position corner
  weights on the host.  The combine runs as large DVE tensor_tensor ops in
  2x mode: weights are stored pair-duplicated ([w,w] adjacent) so the
  broadcast AP keeps a packed innermost [1,2] dim; corner partials reduce
  with bf16 adds into a bf16 accumulator (host upcasts to f32).  All psum
  evictions run on ACT so the Vector engine stays dedicated to the combine;
  table groups build in widths 1/2/3/3 so gathers start early; the last tap
  gathers in halves so its combine and the output DMA overlap.  The output
  stays position-major on device; the host does the final [j,f] -> [f,j]
  transpose.

Shapes (hardcoded per spec): x (8,128,64,64) f32, offset (8,18,64,64),
mask (8,9,64,64), weight (128,128,3,3), out (8,128,64,64) f32.
"""

import numpy as np
import ml_dtypes
from contextlib import ExitStack

import concourse.bass as bass
import concourse.bacc as bacc
import concourse.tile as tile
from concourse import mybir
from concourse.bass_utils import run_bass_kernel_spmd

B, C, H, W = 8, 128, 64, 64
F = 128
KH = KW = 3
KK = KH * KW
HW = H * W  # 4096
NP = 128
NII = HW // NP  # 32 position blocks
NTT = 33  # table row tiles (t in [0, 4224)); gather uses rows [0, 4160]
TROWS = NTT * NP
TCOLS = 4 * F  # 512
# column-slot source shifts: TC[t] = [Y(t-65) | Y(t-64) | Y(t-1) | Y(t)]
SLOT_SHIFT = (-65, -64, -1, 0)
XPAD_LO = 65  # x padding so shifted tiles never index out of range
XPAD = XPAD_LO + TROWS + 64  # padded x columns

# gather call schedule: (base_tap, [(tap, ii0, nii), ...]); idx values are
# stored relative to base_tap's table rows (int16-safe: max 1*TROWS+4160).
# 6144-idx calls amortize the ~9us fixed SWDGE cost per dma_gather; the
# final 2048-idx call keeps the critical tail short.
G_CALLS = [(k, [(k, 0, 32)]) for k in range(8)] + [
    (8, [(8, 0, 16)]),
    (8, [(8, 16, 8)]),
    (8, [(8, 24, 8)]),
]

BF16 = mybir.dt.bfloat16
F32 = mybir.dt.float32
I16 = mybir.dt.int16


def _prep_indices_weights(offset, mask):
    """Per-sample host prep. offset [18,H,W], mask [9,H,W] ->
    idx int16 [128, KK*256], wts bf16 [128, KK*NII*4*2]."""
    off = offset.reshape(KK, 2, H, W)
    dy, dx = off[:, 0], off[:, 1]
    ki, kj = np.meshgrid(np.arange(KH), np.arange(KW), indexing="ij")
    ki = ki.reshape(KK, 1, 1).astype(np.float32)
    kj = kj.reshape(KK, 1, 1).astype(np.float32)
    base_y = (np.arange(H, dtype=np.float32) - 1.0)[None, :, None] + ki
    base_x = (np.arange(W, dtype=np.float32) - 1.0)[None, None, :] + kj
    py = base_y + dy
    px = base_x + dx
    y0 = np.floor(py)
    x0 = np.floor(px)
    ly = (py - y0).astype(np.float32)
    lx = (px - x0).astype(np.float32)
    hy = 1.0 - ly
    hx = 1.0 - lx
    y0i = y0.astype(np.int64)
    x0i = x0.astype(np.int64)

    vy0 = (y0i >= 0) & (y0i < H)
    vy1 = (y0i + 1 >= 0) & (y0i + 1 < H)
    vx0 = (x0i >= 0) & (x0i < W)
    vx1 = (x0i + 1 >= 0) & (x0i + 1 < W)

    m = mask.reshape(KK, H, W)
    # corner order matches table slots: (y0,x0) (y0,x1) (y1,x0) (y1,x1)
    w4 = np.stack(
        [
            (hy * hx * m * (vy0 & vx0)).reshape(KK, HW),
            (hy * lx * m * (vy0 & vx1)).reshape(KK, HW),
            (ly * hx * m * (vy1 & vx0)).reshape(KK, HW),
            (ly * lx * m * (vy1 & vx1)).reshape(KK, HW),
        ],
        axis=1,
    ).astype(np.float32)  # [KK, 4, HW]

    flat = np.clip(y0i * W + x0i + 65, 0, HW + 64).reshape(KK, HW)

    # idx: per gather call, ordinals o wrapped -> [o%16, o//16], replicated
    # to 128 partitions (dma_gather consumes idxs from each 16-partition
    # group).  Rows of taps beyond the call's base tap get +TROWS offsets.
    cols = []
    for kk0, parts in G_CALLS:
        fc = np.concatenate(
            [
                flat[kk][ii0 * NP : (ii0 + nii) * NP] + (kk - kk0) * TROWS
                for kk, ii0, nii in parts
            ]
        )
        wrapped = fc.astype(np.int16).reshape(-1, 16).T
        cols.append(np.tile(wrapped, (8, 1)))
    idx_dev = np.concatenate(cols, axis=1)  # [128, KK*256]

    # wts: [p, kk, ii, cr, 2] with value w4[kk, cr, ii*128+p] pair-duplicated
    wtsp = w4.reshape(KK, 4, NII, NP).transpose(3, 0, 2, 1)  # [p, kk, ii, cr]
    wts_dev = np.repeat(
        np.ascontiguousarray(wtsp).astype(ml_dtypes.bfloat16)[..., None], 2, axis=-1
    )
    return idx_dev, wts_dev.reshape(NP, KK * NII * 4 * 2)


def _split_overfull_waits(nc):
    """This walrus build accepts 1 sync-wait per instruction (2 for EVSEM).
    Move extras onto preceding same-engine NoOps."""
    for f in nc.m.functions:
        for bb in f.blocks:
            new_list = []
            for ins in bb.instructions:
                si = ins.sync_info
                waits = list(si.on_wait) if si and si.on_wait else []
                cap = 2 if isinstance(ins, mybir.InstEventSemaphore) else 1
                if len(waits) > cap:
                    extra, keep = waits[:-cap], waits[-cap:]
                    for k, w in enumerate(extra):
                        nop = mybir.InstNoOp(
                            name=f"{ins.name}_waitsplit{k}",
                            sync_info=mybir.SyncInfo(on_wait=[w], on_update=[]),
                            bass_nofuse=True,
                            engine=ins.engine,
                        )
                        new_list.append(nop)
                        nc.register_instruction(nop, overwrite=True)
                    si.on_wait = keep
                new_list.append(ins)
            bb.instructions[:] = new_list


def _build_nc():
    nc = bacc.Bacc(None, target_bir_lowering=False, debug=False)
    x_d = nc.dram_tensor("x", [NP, XPAD], BF16, kind="ExternalInput")
    wt_d = nc.dram_tensor("wt", [NP, KK * F], BF16, kind="ExternalInput")
    idx_d = nc.dram_tensor("idx", [NP, KK * 256], I16, kind="ExternalInput")
    wts_d = nc.dram_tensor("wts", [NP, KK * NII * 4 * 2], BF16, kind="ExternalInput")
    out_d = nc.dram_tensor("out", [NP, HW], BF16, kind="ExternalOutput")
    tbl_d = nc.dram_tensor("tbl", [KK, TROWS, TCOLS], BF16, kind="Internal")

    TBL_KK = TROWS * TCOLS

    with tile.TileContext(nc) as tc, ExitStack() as ctx:
        cpool = ctx.enter_context(tc.tile_pool(name="const", bufs=1))
        tcst_pool = ctx.enter_context(tc.tile_pool(name="tcst", bufs=4))
        gpool = ctx.enter_context(tc.tile_pool(name="gat", bufs=3))
        mpool = ctx.enter_context(tc.tile_pool(name="mul", bufs=1))
        spool = ctx.enter_context(tc.tile_pool(name="sum", bufs=1))
        accpool = ctx.enter_context(tc.tile_pool(name="acc", bufs=1))
        pspool = ctx.enter_context(tc.tile_pool(name="ps", bufs=2, space="PSUM"))

        x_sb = cpool.tile([NP, XPAD], BF16)
        wt_sb = cpool.tile([NP, KK * F], BF16)
        idx_sb = cpool.tile([NP, KK * 256], I16)
        wts_sb = cpool.tile([NP, KK, NII, 4, 2], BF16)
        acc_sb = accpool.tile([NP, NII, NP], BF16)

        # split x load so tap-0's first table matmuls start sooner
        nc.sync.dma_start(x_sb[:, 0:1152], x_d[:, 0:1152])
        nc.sync.dma_start(wt_sb[:], wt_d[:])
        nc.sync.dma_start(x_sb[:, 1152:], x_d[:, 1152:])
        nc.sync.dma_start(idx_sb[:], idx_d[:])
        nc.sync.dma_start(wts_sb[:].rearrange("p a b c d -> p (a b c d)"), wts_d[:])

        # ---- Stage A: build TC tables.
        # per (group, tt): stationary = shifted x tile; 4 slot matmuls
        # -> psum f32 -> ACT evict (cast bf16) into tcst staging
        # [q, (kk, slot, f)]; one DMA per (group, tt) writes the group's rows.
        # First group = 1 tap so gathers start early; rest in widths of 2.
        for lo, hi in ((0, 1), (1, 2), (2, 3), (3, 5), (5, 7), (7, 9)):
            nk = hi - lo
            if nk == 1:
                # width-1 groups: stage 2 tt per write DMA (fewer sync
                # dispatches + fewer sems on the dependent gather)
                for tt0 in range(0, NTT, 2):
                    npair = min(2, NTT - tt0)
                    tcst = tcst_pool.tile([NP, 2, 4, F], BF16, tag="tc1")
                    for ti in range(npair):
                        tt = tt0 + ti
                        ps = pspool.tile([NP, 4, 512], F32)
                        for s in range(4):
                            xoff = XPAD_LO + tt * NP + SLOT_SHIFT[s]
                            nc.tensor.matmul(
                                ps[:, s, 0:F],
                                x_sb[:, xoff : xoff + NP],
                                wt_sb[:, lo * F : hi * F],
                                start=True,
                                stop=True,
                            )
                        # tap-0 evictions ride on DVE (provably idle: the
                        # first combine cannot start before tap-0's gather
                        # finishes); ACT handles later groups.
                        if lo == 0:
                            nc.vector.tensor_copy(tcst[:, ti], ps[:, :, 0:F])
                        else:
                            nc.scalar.copy(tcst[:, ti], ps[:, :, 0:F])
                    dst = bass.AP(
                        tbl_d,
                        lo * TBL_KK + tt0 * NP * TCOLS,
                        [[TCOLS, NP], [NP * TCOLS, npair], [1, TCOLS]],
                    )
                    nc.sync.dma_start(dst, tcst[:, 0:npair])
                continue
            for tt in range(NTT):
                tcst = tcst_pool.tile([NP, nk, 4, F], BF16, tag=f"tc{nk}")
                ps = pspool.tile([NP, 4, 512], F32)
                for s in range(4):
                    xoff = XPAD_LO + tt * NP + SLOT_SHIFT[s]
                    nc.tensor.matmul(
                        ps[:, s, 0 : nk * F],
                        x_sb[:, xoff : xoff + NP],
                        wt_sb[:, lo * F : hi * F],
                        start=True,
                        stop=True,
                    )
                src = ps[:, :, 0 : nk * F].rearrange("p s (k f) -> p s k f", k=nk)
                dst_r = tcst[:].rearrange("p k s f -> p s k f")
                nc.scalar.copy(dst_r, src)
                dst = bass.AP(
                    tbl_d,
                    lo * TBL_KK + tt * NP * TCOLS,
                    [[TCOLS, NP], [TBL_KK, nk], [1, TCOLS]],
                )
                nc.sync.dma_start(dst, tcst[:])

        # ---- Stage B: gather + batched combine per G_CALLS schedule.
        def combine(kk, g_t, ii0, nii):
            """g_t [NP, nii, 4, F] gathered corners for positions
            (ii0..ii0+nii)*128; weighted-reduce into acc_sb."""
            w_ap = (
                wts_sb[:, kk, ii0 : ii0 + nii]
                .rearrange("p i c d -> p (i c) d")[:, :, None, :]
                .to_broadcast([NP, nii * 4, F // 2, 2])
            )
            if nii <= 16:
                # tail calls: separate mult output so the add tree reads
                # DVE-produced data (no DMA-sem waits -> shorter tail)
                m_tile = mpool.tile([NP, 16, 4, F], BF16, tag="m_t", name="m_tile")
                m_t = m_tile[:, :nii]
            else:
                m_t = g_t
            nc.vector.tensor_mul(
                m_t.rearrange("p i c (a b) -> p (i c) a b", b=2),
                g_t.rearrange("p i c (a b) -> p (i c) a b", b=2),
                w_ap,
            )
            g_t = m_t
            a_sl = acc_sb[:, ii0 : ii0 + nii]
            t01 = spool.tile([NP, NII, F], BF16, tag="t01")
            t23 = spool.tile([NP, NII, F], BF16, tag="t23")
            nc.vector.tensor_add(t01[:, :nii], g_t[:, :, 0], g_t[:, :, 1])
            nc.vector.tensor_add(t23[:, :nii], g_t[:, :, 2], g_t[:, :, 3])
            if kk == 0:
                nc.vector.tensor_add(a_sl, t01[:, :nii], t23[:, :nii])
            else:
                nc.vector.tensor_add(t01[:, :nii], t01[:, :nii], t23[:, :nii])
                nc.vector.tensor_add(a_sl, a_sl, t01[:, :nii])

        off = 0
        for ci, (kk0, parts) in enumerate(G_CALLS):
            nb = sum(nii for _, _, nii in parts)
            n = nb * NP
            g_full = gpool.tile([NP, NII, 4, F], BF16, tag="g_t")
            g_t = g_full[:, 0:nb]
            span = (max(kk for kk, _, _ in parts) - kk0) * TROWS + HW + 65
            src = bass.AP(tbl_d, kk0 * TBL_KK, [[TCOLS, span], [1, TCOLS]])
            nc.gpsimd.dma_gather(
                out_ap=g_t.rearrange("p i c f -> p i (c f)"),
                in_ap=src,
                idxs_ap=idx_sb[:, off : off + n // 16],
                num_idxs=n,
                num_idxs_reg=n,
                elem_size=TCOLS,
                single_packet=False,
            )
            off += n // 16
            pos = 0
            for kk, ii0, nii in parts:
                combine(kk, g_t[:, pos : pos + nii], ii0, nii)
                pos += nii
            if ci == len(G_CALLS) - 3:
                # all taps' combines for positions ii<16 are done
                nc.sync.dma_start(
                    out_d[:, 0 : NP * 16],
                    acc_sb[:, 0:16].rearrange("p i f -> p (i f)"),
                )
            elif ci == len(G_CALLS) - 2:
                nc.sync.dma_start(
                    out_d[:, NP * 16 : NP * 24],
                    acc_sb[:, 16:24].rearrange("p i f -> p (i f)"),
                )
            elif ci == len(G_CALLS) - 1:
                nc.sync.dma_start(
                    out_d[:, NP * 24 :],
                    acc_sb[:, 24:].rearrange("p i f -> p (i f)"),
                )

    nc.compile()
    _split_overfull_waits(nc)
    return nc


_NC_CACHE = {}


def _get_nc():
    if "nc" not in _NC_CACHE:
        _NC_CACHE["nc"] = _build_nc()
    return _NC_CACHE["nc"]


def _prep_x(xb):
    """x [C,H,W] f32 -> padded bf16 [128, XPAD]."""
    xp = np.zeros((C, XPAD), ml_dtypes.bfloat16)
    xp[:, XPAD_LO : XPAD_LO + HW] = xb.reshape(C, HW).astype(ml_dtypes.bfloat16)
    return xp


def kernel(x, offset, mask, weight, **run_kwargs):
    x = np.asarray(x, np.float32)
    offset = np.asarray(offset, np.float32)
    mask = np.asarray(mask, np.float32)
    weight = np.asarray(weight, np.float32)

    wt = np.transpose(weight.reshape(F, C, KK), (1, 2, 0)).reshape(C, KK * F)
    wt = np.ascontiguousarray(wt).astype(ml_dtypes.bfloat16)

    in_maps = []
    for b in range(B):
        idx_dev, wts_dev = _prep_indices_weights(offset[b], mask[b])
        in_maps.append(
            {
                "x": _prep_x(x[b]),
                "wt": wt,
                "idx": idx_dev,
                "wts": wts_dev,
            }
        )

    nc = _get_nc()
    res = run_bass_kernel_spmd(nc, in_maps, core_ids=list(range(8)), **run_kwargs)
    out = np.stack(
        [
            np.asarray(res.results[b]["out"])
            .astype(np.float32)
            .reshape(NP, NII, F)
            .transpose(2, 1, 0)
            .reshape(F, H, W)
            for b in range(B)
        ]
    )
    if run_kwargs:
        kernel.last_results = res
    return out
